# revision 4
# baseline (speedup 1.0000x reference)
"""RNN-T Joiner kernel for Trainium2 (Bass/Tile), 8-core data-parallel over batch.

out[b,t,u,v] = (enc[b,t] @ We)[v] + (pred[b,u] @ Wp)[v] + bias[v]

Strategy per core (one batch element):
  - PE (fp32): enc_proj [256,1024] and pred_b [65,1024] projections.
  - PE (fp32r, full rate): broadcast pred_b rows across the 128 t-partitions
    via one-hot selection matmuls into PSUM.
  - DVE: one tensor_tensor add per output element (the mandatory PSUM->SBUF
    trip) producing staged output tiles.
  - HWDGE DMA: ~4 MB contiguous stores of the [T,U,V] lattice.
"""

import sys

sys.path.insert(0, "/opt/trn_rl_repo")

import numpy as np

B, T, U1, D, V = 8, 256, 65, 640, 1024
KC = D // 128  # 5 contraction chunks
UBLK = 8       # u's per output DMA block (8 full blocks + 1 tail u)
NBLK = U1 // UBLK  # 8

_COMPILED = None


def _build():
    import concourse.bacc as bacc
    import concourse.tile as tile
    import concourse.mybir as mybir
    from concourse.bass_utils import run_bass_kernel_spmd  # noqa: F401 (import check)

    f32 = mybir.dt.float32
    f32r = mybir.dt.float32r  # noqa: F841
    bf16 = mybir.dt.bfloat16

    nc = bacc.Bacc("TRN2", target_bir_lowering=False, debug=False, num_devices=8)

    encT = nc.dram_tensor("encT", [D, T], f32, kind="ExternalInput")
    predT = nc.dram_tensor("predT", [D, U1], f32, kind="ExternalInput")
    W = nc.dram_tensor("W", [2 * D, V], f32, kind="ExternalInput")
    bias = nc.dram_tensor("bias", [1, V], f32, kind="ExternalInput")
    ones = nc.dram_tensor("ones", [1, 128], f32, kind="ExternalInput")
    sel = nc.dram_tensor("sel", [U1, U1 * 128], bf16, kind="ExternalInput")
    out = nc.dram_tensor("out", [T, U1 * V], f32, kind="ExternalOutput")

    with tile.TileContext(nc) as tc:
        with tc.tile_pool(name="consts", bufs=1) as cp:
            encT_sb = []
            predT_sb = []
            We_sb = []
            Wp_sb = []
            for c in range(KC):
                t_ = cp.tile([128, T], f32, tag=f"encT{c}")
                nc.sync.dma_start(t_[:], encT[c * 128:(c + 1) * 128, :])
                encT_sb.append(t_)
                t_ = cp.tile([128, U1], f32, tag=f"predT{c}")
                nc.sync.dma_start(t_[:], predT[c * 128:(c + 1) * 128, :])
                predT_sb.append(t_)
                t_ = cp.tile([128, V], f32, tag=f"We{c}")
                nc.sync.dma_start(t_[:], W[c * 128:(c + 1) * 128, :])
                We_sb.append(t_)
                t_ = cp.tile([128, V], f32, tag=f"Wp{c}")
                nc.sync.dma_start(t_[:], W[D + c * 128:D + (c + 1) * 128, :])
                Wp_sb.append(t_)
            bias_sb = cp.tile([1, V], f32, tag="bias")
            nc.sync.dma_start(bias_sb[:], bias[:])
            ones_sb = cp.tile([1, 128], f32, tag="ones")
            nc.sync.dma_start(ones_sb[:], ones[:])
            sel_sb = cp.tile([U1, U1 * 128], bf16, tag="sel")
            nc.sync.dma_start(sel_sb[:], sel[:])

            pred_b_sb = cp.tile([U1, V], bf16, tag="pred_b")
            enc_dup = []
            for tt in range(2):
                t_ = cp.tile([128, 2 * V], f32, tag=f"enc_dup{tt}")
                enc_dup.append(t_)

            # ---- setup: projections (fp32 PE matmuls) ----
            with tc.tile_pool(name="spsum", bufs=2, space="PSUM") as sp:
                ps_p = sp.tile([U1, V], f32, tag="ps")
                for vt in range(2):
                    vs = slice(vt * 512, (vt + 1) * 512)
                    for c in range(KC):
                        nc.tensor.matmul(
                            ps_p[:, vs], predT_sb[c][:, :U1], Wp_sb[c][:, vs],
                            start=(c == 0), stop=False)
                    nc.tensor.matmul(
                        ps_p[:, vs], ones_sb[0:1, 0:U1], bias_sb[0:1, vs],
                        start=False, stop=True)
                nc.vector.tensor_copy(pred_b_sb[:], ps_p[:])

                for tt in range(2):
                    ts_ = slice(tt * 128, (tt + 1) * 128)
                    ps_e = sp.tile([128, V], f32, tag="pse")
                    for vt in range(2):
                        vs = slice(vt * 512, (vt + 1) * 512)
                        for c in range(KC):
                            nc.tensor.matmul(
                                ps_e[:, vs], encT_sb[c][:, ts_], We_sb[c][:, vs],
                                start=(c == 0), stop=(c == KC - 1))
                    nc.vector.tensor_copy(enc_dup[tt][:, 0:V], ps_e[:])
                    nc.vector.tensor_copy(enc_dup[tt][:, V:2 * V], ps_e[:])

            sel_r = sel_sb
            predb_r = pred_b_sb

            # ---- main loop: broadcast-add-store ----
            with tc.tile_pool(name="outp", bufs=2) as op_, \
                 tc.tile_pool(name="mpsum", bufs=2, space="PSUM") as mp:
                for tt in range(2):
                    rs = slice(tt * 128, (tt + 1) * 128)
                    for blk in range(NBLK):
                        stage = op_.tile([128, UBLK * V], f32, tag="stage")
                        for pair in range(UBLK // 2):
                            u0 = blk * UBLK + 2 * pair
                            ps = mp.tile([128, 2048], f32, tag="mps")
                            for j in range(4):
                                u = u0 + j // 2
                                vt = j % 2
                                nc.tensor.matmul(
                                    ps[:, j * 512:(j + 1) * 512],
                                    sel_r[0:U1, u * 128:(u + 1) * 128],
                                    predb_r[0:U1, vt * 512:(vt + 1) * 512],
                                    start=True, stop=True)
                            nc.vector.tensor_add(
                                stage[:, pair * 2048:(pair + 1) * 2048],
                                enc_dup[tt][:], ps[:])
                        nc.sync.dma_start(
                            out[rs, blk * UBLK * V:(blk + 1) * UBLK * V], stage[:])
                    # tail u = 64
                    u = U1 - 1
                    stage = op_.tile([128, UBLK * V], f32, tag="stage")
                    ps = mp.tile([128, 2048], f32, tag="mps")
                    for vt in range(2):
                        nc.tensor.matmul(
                            ps[:, vt * 512:(vt + 1) * 512],
                            sel_r[0:U1, u * 128:(u + 1) * 128],
                            predb_r[0:U1, vt * 512:(vt + 1) * 512],
                            start=True, stop=True)
                    nc.vector.tensor_add(
                        stage[:, 0:V], enc_dup[tt][:, 0:V], ps[:, 0:V])
                    nc.sync.dma_start(out[rs, u * V:(u + 1) * V], stage[:, 0:V])

    nc.compile()
    return nc


def _get_compiled():
    global _COMPILED
    if _COMPILED is None:
        _COMPILED = _build()
    return _COMPILED


def _in_maps(encoder_out, predictor_out, W, b):
    import ml_dtypes
    sel = np.zeros((U1, U1 * 128), dtype=ml_dtypes.bfloat16)
    for u in range(U1):
        sel[u, u * 128:(u + 1) * 128] = 1.0
    ones = np.ones((1, 128), dtype=np.float32)
    bias = np.ascontiguousarray(b.reshape(1, V).astype(np.float32))
    Wc = np.ascontiguousarray(W.astype(np.float32))
    maps = []
    for i in range(B):
        maps.append({
            "encT": np.ascontiguousarray(encoder_out[i].T.astype(np.float32)),
            "predT": np.ascontiguousarray(predictor_out[i].T.astype(np.float32)),
            "W": Wc,
            "bias": bias,
            "ones": ones,
            "sel": sel,
        })
    return maps


def run(encoder_out, predictor_out, W, b, trace=False, tmpdir=None):
    from concourse.bass_utils import run_bass_kernel_spmd

    nc = _get_compiled()
    maps = _in_maps(encoder_out, predictor_out, W, b)
    res = run_bass_kernel_spmd(
        nc, maps, list(range(B)), trace=trace,
        **({"tmpdir": tmpdir} if tmpdir else {}))
    outs = np.stack([res.results[i]["out"].reshape(T, U1, V) for i in range(B)])
    return outs, res


def kernel(encoder_out, predictor_out, W, b):
    outs, _ = run(encoder_out, predictor_out, W, b)
    return outs


# revision 5
# speedup vs baseline: 1.0259x; 1.0259x over previous
"""RNN-T Joiner kernel for Trainium2 (Bass/Tile), 8-core data-parallel over batch.

out[b,t,u,v] = (enc[b,t] @ We)[v] + (pred[b,u] @ Wp)[v] + bias[v]

Strategy per core (one batch element):
  - PE (fp32): enc_proj [256,1024] and pred_b [65,1024] projections.
  - PE (fp32r, full rate): broadcast pred_b rows across the 128 t-partitions
    via one-hot selection matmuls into PSUM.
  - DVE: one tensor_tensor add per output element (the mandatory PSUM->SBUF
    trip) producing staged output tiles.
  - HWDGE DMA: ~4 MB contiguous stores of the [T,U,V] lattice.
"""

import sys

sys.path.insert(0, "/opt/trn_rl_repo")

import numpy as np

B, T, U1, D, V = 8, 256, 65, 640, 1024
KC = D // 128  # 5 contraction chunks
UBLK = 4       # u's per output DMA block (16 full blocks + 1 tail u)
NBLK = U1 // UBLK  # 8

_COMPILED = None


def _build():
    import concourse.bacc as bacc
    import concourse.tile as tile
    import concourse.mybir as mybir
    from concourse.bass_utils import run_bass_kernel_spmd  # noqa: F401 (import check)

    f32 = mybir.dt.float32
    f32r = mybir.dt.float32r  # noqa: F841
    bf16 = mybir.dt.bfloat16

    nc = bacc.Bacc("TRN2", target_bir_lowering=False, debug=False, num_devices=8)

    encT = nc.dram_tensor("encT", [D, T], f32, kind="ExternalInput")
    predT = nc.dram_tensor("predT", [D, U1], f32, kind="ExternalInput")
    W = nc.dram_tensor("W", [2 * D, V], f32, kind="ExternalInput")
    bias = nc.dram_tensor("bias", [1, V], f32, kind="ExternalInput")
    ones = nc.dram_tensor("ones", [1, 128], f32, kind="ExternalInput")
    sel = nc.dram_tensor("sel", [U1, U1 * 128], f32r, kind="ExternalInput")
    out = nc.dram_tensor("out", [T, U1 * V], f32, kind="ExternalOutput")

    with tile.TileContext(nc) as tc:
        with tc.tile_pool(name="consts", bufs=1) as cp:
            encT_sb = []
            predT_sb = []
            We_sb = []
            Wp_sb = []
            for c in range(KC):
                t_ = cp.tile([128, T], f32, tag=f"encT{c}")
                nc.sync.dma_start(t_[:], encT[c * 128:(c + 1) * 128, :])
                encT_sb.append(t_)
                t_ = cp.tile([128, U1], f32, tag=f"predT{c}")
                nc.sync.dma_start(t_[:], predT[c * 128:(c + 1) * 128, :])
                predT_sb.append(t_)
                t_ = cp.tile([128, V], f32, tag=f"We{c}")
                nc.sync.dma_start(t_[:], W[c * 128:(c + 1) * 128, :])
                We_sb.append(t_)
                t_ = cp.tile([128, V], f32, tag=f"Wp{c}")
                nc.sync.dma_start(t_[:], W[D + c * 128:D + (c + 1) * 128, :])
                Wp_sb.append(t_)
            bias_sb = cp.tile([1, V], f32, tag="bias")
            nc.sync.dma_start(bias_sb[:], bias[:])
            ones_sb = cp.tile([1, 128], f32, tag="ones")
            nc.sync.dma_start(ones_sb[:], ones[:])
            sel_sb = cp.tile([U1, U1 * 128], f32r, tag="sel")
            nc.sync.dma_start(sel_sb[:], sel[:])

            pred_b_sb = cp.tile([U1, V], f32r, tag="pred_b")
            enc_dup = []
            for tt in range(2):
                t_ = cp.tile([128, 2 * V], f32, tag=f"enc_dup{tt}")
                enc_dup.append(t_)

            # ---- setup: projections (fp32 PE matmuls) ----
            with tc.tile_pool(name="spsum", bufs=2, space="PSUM") as sp:
                ps_p = sp.tile([U1, V], f32, tag="ps")
                for vt in range(2):
                    vs = slice(vt * 512, (vt + 1) * 512)
                    for c in range(KC):
                        nc.tensor.matmul(
                            ps_p[:, vs], predT_sb[c][:, :U1], Wp_sb[c][:, vs],
                            start=(c == 0), stop=False)
                    nc.tensor.matmul(
                        ps_p[:, vs], ones_sb[0:1, 0:U1], bias_sb[0:1, vs],
                        start=False, stop=True)
                nc.vector.tensor_copy(pred_b_sb[:], ps_p[:])

                for tt in range(2):
                    ts_ = slice(tt * 128, (tt + 1) * 128)
                    ps_e = sp.tile([128, V], f32, tag="pse")
                    for vt in range(2):
                        vs = slice(vt * 512, (vt + 1) * 512)
                        for c in range(KC):
                            nc.tensor.matmul(
                                ps_e[:, vs], encT_sb[c][:, ts_], We_sb[c][:, vs],
                                start=(c == 0), stop=(c == KC - 1))
                    nc.vector.tensor_copy(enc_dup[tt][:, 0:V], ps_e[:])
                    nc.vector.tensor_copy(enc_dup[tt][:, V:2 * V], ps_e[:])

            sel_r = sel_sb
            predb_r = pred_b_sb

            # ---- main loop: broadcast-add-store ----
            with tc.tile_pool(name="outp", bufs=4) as op_, \
                 tc.tile_pool(name="mpsum", bufs=2, space="PSUM") as mp:
                for tt in range(2):
                    rs = slice(tt * 128, (tt + 1) * 128)
                    for blk in range(NBLK):
                        stage = op_.tile([128, UBLK * V], f32, tag="stage")
                        for pair in range(UBLK // 2):
                            u0 = blk * UBLK + 2 * pair
                            ps = mp.tile([128, 2048], f32, tag="mps")
                            for j in range(4):
                                u = u0 + j // 2
                                vt = j % 2
                                nc.tensor.matmul(
                                    ps[:, j * 512:(j + 1) * 512],
                                    sel_r[0:U1, u * 128:(u + 1) * 128],
                                    predb_r[0:U1, vt * 512:(vt + 1) * 512],
                                    start=True, stop=True)
                            nc.vector.tensor_add(
                                stage[:, pair * 2048:(pair + 1) * 2048],
                                enc_dup[tt][:], ps[:])
                        nc.sync.dma_start(
                            out[rs, blk * UBLK * V:(blk + 1) * UBLK * V], stage[:])
                    # tail u = 64
                    u = U1 - 1
                    stage = op_.tile([128, UBLK * V], f32, tag="stage")
                    ps = mp.tile([128, 2048], f32, tag="mps")
                    for vt in range(2):
                        nc.tensor.matmul(
                            ps[:, vt * 512:(vt + 1) * 512],
                            sel_r[0:U1, u * 128:(u + 1) * 128],
                            predb_r[0:U1, vt * 512:(vt + 1) * 512],
                            start=True, stop=True)
                    nc.vector.tensor_add(
                        stage[:, 0:V], enc_dup[tt][:, 0:V], ps[:, 0:V])
                    nc.sync.dma_start(out[rs, u * V:(u + 1) * V], stage[:, 0:V])

    nc.compile()
    return nc


def _get_compiled():
    global _COMPILED
    if _COMPILED is None:
        _COMPILED = _build()
    return _COMPILED


def _in_maps(encoder_out, predictor_out, W, b):
    sel = np.zeros((U1, U1 * 128), dtype=np.float32)
    for u in range(U1):
        sel[u, u * 128:(u + 1) * 128] = 1.0
    ones = np.ones((1, 128), dtype=np.float32)
    bias = np.ascontiguousarray(b.reshape(1, V).astype(np.float32))
    Wc = np.ascontiguousarray(W.astype(np.float32))
    maps = []
    for i in range(B):
        maps.append({
            "encT": np.ascontiguousarray(encoder_out[i].T.astype(np.float32)),
            "predT": np.ascontiguousarray(predictor_out[i].T.astype(np.float32)),
            "W": Wc,
            "bias": bias,
            "ones": ones,
            "sel": sel,
        })
    return maps


def run(encoder_out, predictor_out, W, b, trace=False, tmpdir=None):
    from concourse.bass_utils import run_bass_kernel_spmd

    nc = _get_compiled()
    maps = _in_maps(encoder_out, predictor_out, W, b)
    res = run_bass_kernel_spmd(
        nc, maps, list(range(B)), trace=trace,
        **({"tmpdir": tmpdir} if tmpdir else {}))
    outs = np.stack([res.results[i]["out"].reshape(T, U1, V) for i in range(B)])
    return outs, res


def kernel(encoder_out, predictor_out, W, b):
    outs, _ = run(encoder_out, predictor_out, W, b)
    return outs
